# revision 1
# baseline (speedup 1.0000x reference)
"""Trainium2 Bass kernel for nn_BagModel (segment_reduce).

Model: h = relu(x @ W1 + b1); bag_feat = segment_mean(h, ids); out = bag_feat @ W2 + b2
  x [262144, 1024] f32, ids [262144] int64 (sorted, 512 bags), W1 [1024, 512],
  b1 [512], W2 [512, 2], b2 [2]  ->  out [512, 2] f32

Strategy (8 NeuronCores, data-parallel over equal row ranges):
  - Host: split rows EQUALLY across cores (262144/8 = 64 macrotiles exactly,
    zero padding). Bags straddling a core boundary produce partial logits on
    both cores; logits are linear in the bag sums (1/count and W2 folded
    host-side into w2b[b, c*512+j] = W2[j,c]/count[b], b2 masked to the
    owning core), so the host overlap-adds per-core outputs. Pre-transpose
    each shard to xT [1024, L] bf16 so each 128-row subtile's x-chunk is
    directly the stationary (lhsT) matmul operand; build a one-hot
    row->local-bag-slot selection matrix per subtile.
  - Device, per 128-row subtile: 8 accumulating matmuls (x chunks x W1 chunks)
    into PSUM; DVE adds a row-broadcast b1; ScalarE relu-copies to bf16.
    Segment sums = one-hot matmuls (Sel^T @ h, Sel exact {0,1} in bf16)
    accumulating into a single PSUM tile that lives across the whole kernel,
    batched every 4 macrotiles to minimize stationary-switch stalls on the PE.
  - Epilogue (per core): logits[b, c] = b2[c] + sum_j sums[b, j]*w2b[b, c, j]
    via DVE multiply + ScalarE accum_out reduction - no transpose anywhere.
    DMA [64, 2] f32 out; host concatenates the 8 core outputs.
  Numerics: bf16 inputs with f32 accumulation everywhere; measured rel err vs
  the f32 reference = 3.2e-4 (absmax-relative).
"""

import numpy as np
import ml_dtypes

N_BAGS = 512
N_CORES = 8
BPC = N_BAGS // N_CORES  # bags per core
D_IN = 1024
D_H = 512
KCH = D_IN // 128  # k-chunks of the contraction dim
MACRO = 512  # rows per macrotile (one x DMA)
SUB = 128  # rows per subtile (one PSUM tile)

_BF16 = ml_dtypes.bfloat16


def _build_nc(n_macro: int, b2vals):
    import concourse.bacc as bacc
    import concourse.mybir as mybir
    from concourse.tile import TileContext

    f32 = mybir.dt.float32
    bf16 = mybir.dt.bfloat16
    RELU = mybir.ActivationFunctionType.Relu
    COPY = mybir.ActivationFunctionType.Copy

    nc = bacc.Bacc(None, target_bir_lowering=False)
    L = n_macro * MACRO
    xT = nc.dram_tensor("xT", [D_IN, L], bf16, kind="ExternalInput")
    # sel one-hot padded to 128 bag-columns per subtile (cols BPC..127 are
    # zero -> rows BPC..127 of the sums PSUM tile accumulate exact zeros);
    # full-width 128-col stationary keeps the LDWEIGHTS path fast
    sel = nc.dram_tensor("sel", [n_macro, SUB, 4 * SUB], bf16, kind="ExternalInput")
    w1 = nc.dram_tensor("w1", [D_IN, D_H], bf16, kind="ExternalInput")
    b1 = nc.dram_tensor("b1", [SUB, D_H], f32, kind="ExternalInput")
    # w2b[b, c*D_H + j] = W2[j, c] / count[b]  (mean division folded in;
    # b is a LOCAL bag slot - rows are split equally across cores and the
    # host adds the partial logits of bags straddling a core boundary).
    # Cols 2*D_H + c hold b2[c] masked to the single owning core so the
    # host-side overlap-add applies b2 exactly once per bag.
    w2b = nc.dram_tensor("w2b", [SUB, 2 * D_H + 2], f32, kind="ExternalInput")
    out = nc.dram_tensor("out", [SUB, 2], f32, kind="ExternalOutput")

    with TileContext(nc) as tc:
        with (
            tc.tile_pool(name="const", bufs=1) as cpool,
            tc.tile_pool(name="xp", bufs=4) as xpool,
            tc.tile_pool(name="selp", bufs=6) as selpool,
            tc.tile_pool(name="hp", bufs=6) as hpool,
            tc.tile_pool(name="pp", bufs=4, space="PSUM") as ppool,
            tc.tile_pool(name="sp", bufs=1, space="PSUM") as spool,
        ):
            # constants go on the ACT HWDGE ring (nc.scalar) so the SP ring's
            # sequencer can start issuing x-tile DMAs immediately; odd w1
            # chunks are interleaved with the first x pieces on the SP ring
            # below so both sequencers ramp the first subtile in parallel
            w1_t = cpool.tile([128, KCH * D_H], bf16, name="w1_t")
            for k in range(KCH):
                nc.scalar.dma_start(
                    out=w1_t[:, k * D_H : (k + 1) * D_H],
                    in_=w1[k * 128 : (k + 1) * 128, :],
                )
            # b1 broadcast across all 128 row-partitions (bias add on DVE,
            # keeping the PE stream free of rank-1 bias matmuls)
            b1_t = cpool.tile([SUB, D_H], f32, name="b1_t")
            nc.scalar.dma_start(out=b1_t[:], in_=b1[:])

            sums = spool.tile([SUB, D_H], f32, name="sums")

            pending = []
            first_seg = True
            for m in range(n_macro):
                x_t = xpool.tile([128, KCH * MACRO], bf16, name="x_t")
                # alternate x tiles between the two HWDGE rings so neither
                # sequencer's ~0.65us-per-descriptor issue rate gates the
                # early macrotiles; sel rides the opposite ring
                # first two macrotiles ride the otherwise-idle SP ring so
                # their pieces land ahead of the cold-clock consumption
                xeng, seng = (
                    (nc.sync, nc.scalar)
                    if (m < 2 or m % 2 == 0)
                    else (nc.scalar, nc.sync)
                )
                # first macrotiles in finer pieces so the first matmuls can
                # start as soon as chunk 0 lands (piece sizes matched to the
                # cold-clock consumption rate); halves afterwards
                if m == 0:
                    pieces = [(0, 1), (1, 1), (2, 2), (4, 2), (6, 2)]
                elif m <= 2:
                    pieces = [(0, 2), (2, 2), (4, 2), (6, 2)]
                else:
                    pieces = [(0, 4), (4, 4)]
                for piece, (ks, kstep) in enumerate(pieces):
                    xeng.dma_start(
                        out=x_t[:, ks * MACRO : (ks + kstep) * MACRO].rearrange(
                            "p (k j) -> p k j", j=MACRO
                        ),
                        in_=xT[
                            ks * 128 : (ks + kstep) * 128,
                            m * MACRO : (m + 1) * MACRO,
                        ].rearrange("(k p) j -> p k j", p=128),
                    )
                sel_t = selpool.tile([SUB, 4 * SUB], bf16, name="sel_t")
                seng.dma_start(out=sel_t[:], in_=sel[m])

                for s in range(4):
                    h_ps = ppool.tile([SUB, D_H], f32, name="h_ps")
                    for k in range(KCH):
                        nc.tensor.matmul(
                            h_ps[:],
                            lhsT=x_t[:, k * MACRO + s * SUB : k * MACRO + (s + 1) * SUB],
                            rhs=w1_t[:, k * D_H : (k + 1) * D_H],
                            start=(k == 0),
                            stop=(k == KCH - 1),
                        )
                    hb = hpool.tile([SUB, D_H], f32, name="hb", tag="hb", bufs=4)
                    nc.vector.tensor_add(hb[:], h_ps[:], b1_t[:])
                    h_sb = hpool.tile([SUB, D_H], bf16, name="h_sb", tag="h_sb", bufs=18)
                    nc.scalar.activation(h_sb[:], hb[:], RELU)
                    pending.append((sel_t, s, h_sb))
                # segment matmuls batched across four macrotiles: one
                # stationary-switch region per batch instead of per subtile.
                # Flush points aligned from the END so the final batch is a
                # full 16 - its first segs hide the last subtile's bias/relu
                # chain latency that a short final batch would expose.
                if (n_macro - 1 - m) % 4 == 0:
                    for i, (sel_ref, s, h_sb) in enumerate(pending):
                        nc.tensor.matmul(
                            sums[:],
                            lhsT=sel_ref[:, s * SUB : (s + 1) * SUB],
                            rhs=h_sb[:],
                            start=first_seg,
                            stop=(m == n_macro - 1 and i == len(pending) - 1),
                            skip_group_check=True,
                        )
                        first_seg = False
                    pending = []

            w2b_t = cpool.tile([SUB, 2 * D_H + 2], f32, name="w2b_t")
            nc.scalar.dma_start(out=w2b_t[:], in_=w2b[:])

            logits = cpool.tile([SUB, 2], f32, name="logits")
            for c in range(2):
                # multiply on DVE, row-reduce via ScalarE accum_out so the
                # two class reductions pipeline across engines
                scr = cpool.tile([SUB, D_H], f32, name=f"scr{c}")
                nc.vector.tensor_mul(
                    scr[:], sums[:], w2b_t[:, c * D_H : (c + 1) * D_H]
                )
                scr2 = cpool.tile([SUB, D_H], f32, name=f"scr2{c}")
                red = cpool.tile([SUB, 1], f32, name=f"red{c}")
                nc.scalar.activation(scr2[:], scr[:], COPY, accum_out=red[:])
                nc.vector.tensor_add(
                    logits[:, c : c + 1],
                    red[:],
                    w2b_t[:, 2 * D_H + c : 2 * D_H + c + 1],
                )
            nc.sync.dma_start(out=out[:], in_=logits[:])
    nc.finalize()
    return nc


def _prepare_inputs(x, ids, W1, b1, W2, b2):
    """Equal row split across cores (minimal padding); local bag slots.

    Core k gets rows [k*R, (k+1)*R). A bag straddling a core boundary gets
    partial logits on both cores; since logits are linear in the bag sums
    (the 1/count and W2 factors are folded host-side), the host overlap-adds
    the per-core outputs. b2 rides along masked to the owning core.
    Returns (in_maps, n_macro, first_bag, nloc) for the gather.
    """
    ids = np.asarray(ids).astype(np.int64)
    x = np.asarray(x, dtype=np.float32)
    n = x.shape[0]

    R = -(-n // N_CORES)  # rows per core
    n_macro = max(1, -(-R // MACRO))
    L = n_macro * MACRO

    counts = np.bincount(ids, minlength=N_BAGS).astype(np.float64)
    recip_all = np.where(counts > 0, 1.0 / counts, 0.0).astype(np.float32)
    first_occ = np.searchsorted(ids, np.arange(N_BAGS))  # first row of each bag

    x_bf = x.astype(_BF16)
    w1_bf = np.asarray(W1, dtype=np.float32).astype(_BF16)
    b1_bc = np.ascontiguousarray(
        np.broadcast_to(np.asarray(b1, dtype=np.float32)[None, :], (SUB, D_H))
    )
    W2f = np.asarray(W2, dtype=np.float32)
    b2f = np.asarray(b2, dtype=np.float32).reshape(2)

    in_maps = []
    first_bag = np.zeros(N_CORES, dtype=np.int64)
    nloc = np.zeros(N_CORES, dtype=np.int64)
    for k in range(N_CORES):
        lo, hi = k * R, min((k + 1) * R, n)
        nk = hi - lo
        xT_k = np.zeros((D_IN, L), dtype=_BF16)
        if nk:
            xT_k[:, :nk] = x_bf[lo:hi].T

        g0 = int(ids[lo]) if nk else 0
        first_bag[k] = g0
        sel_k = np.zeros((n_macro, SUB, 4 * SUB), dtype=_BF16)
        if nk:
            r = np.arange(nk)
            lb = ids[lo:hi] - g0  # local bag slot
            assert lb.max() < SUB, "core spans more than 128 bags"
            nloc[k] = int(lb.max()) + 1
            mi = r // MACRO
            pi = r % SUB
            si = (r % MACRO) // SUB
            sel_k[mi, pi, si * SUB + lb] = 1.0

        # local slot b -> global bag g0+b (slots beyond nloc stay zero)
        w2b_k = np.zeros((SUB, 2 * D_H + 2), dtype=np.float32)
        nl = int(nloc[k])
        gl = np.arange(g0, min(g0 + nl, N_BAGS))
        rk = recip_all[gl]  # [nl]
        for c in range(2):
            w2b_k[: len(gl), c * D_H : (c + 1) * D_H] = (
                rk[:, None] * W2f[:, c][None, :]
            )
            # b2 applied only by the core owning the bag's first row
            own = (first_occ[gl] >= lo) & (first_occ[gl] < hi)
            w2b_k[: len(gl), 2 * D_H + c] = np.where(own, b2f[c], 0.0)

        in_maps.append(
            {
                "xT": xT_k,
                "sel": sel_k,
                "w1": w1_bf,
                "b1": b1_bc,
                "w2b": w2b_k,
            }
        )
    return in_maps, n_macro, first_bag, nloc


def _run(x, ids, W1, b1, W2, b2, trace=False, trace_kwargs=None):
    from concourse.bass_utils import run_bass_kernel_spmd

    in_maps, n_macro, first_bag, nloc = _prepare_inputs(x, ids, W1, b1, W2, b2)
    b2f = np.asarray(b2, dtype=np.float32).reshape(2)
    nc = _build_nc(n_macro, b2f)
    res = run_bass_kernel_spmd(
        nc,
        in_maps,
        list(range(N_CORES)),
        trace=trace,
        **(trace_kwargs or {}),
    )
    full = np.zeros((N_BAGS, 2), dtype=np.float32)
    for k in range(N_CORES):
        out_k = np.asarray(res.results[k]["out"], dtype=np.float32)
        g0, nl = int(first_bag[k]), int(nloc[k])
        nl = min(nl, N_BAGS - g0)
        full[g0 : g0 + nl] += out_k[:nl]
    return full, res


def kernel(x, ids, W1, b1, W2, b2):
    out, _ = _run(x, ids, W1, b1, W2, b2, trace=False)
    return out



# revision 2
# speedup vs baseline: 1.7899x; 1.7899x over previous
"""Trainium2 Bass kernel for nn_BagModel (segment_reduce).

Model: h = relu(x @ W1 + b1); bag_feat = segment_mean(h, ids); out = bag_feat @ W2 + b2
  x [262144, 1024] f32, ids [262144] int64 (sorted, 512 bags), W1 [1024, 512],
  b1 [512], W2 [512, 2], b2 [2]  ->  out [512, 2] f32

Strategy (8 NeuronCores, data-parallel over equal row ranges):
  - Host: split rows EQUALLY across cores (262144/8 = 64 macrotiles exactly,
    zero padding). Bags straddling a core boundary produce partial logits on
    both cores; logits are linear in the bag sums (1/count and W2 folded
    host-side into w2b, b2 masked to the owning core), so the host
    overlap-adds per-core outputs. Pre-transpose each shard to xT [1024, L]
    fp8e4 (x scaled by 16; W1 scaled by 256 - both powers of two, folded
    back out via the relu scale and w2b) so each 128-row subtile's x-chunk
    is directly the stationary (lhsT) matmul operand.
  - Device, per 128-row subtile: 4 fp8 DoubleRow accumulating matmuls
    (256-deep k-pairs; 2 fp8 weights/cell -> ~1.8x PE throughput vs bf16)
    into PSUM; DVE adds a row-broadcast 4096*b1; ScalarE relu(x/256) emits
    16*relu(.) as fp8e4 into one half of a subtile-pair tile.
    Segment sums = one-hot DoubleRow matmuls (Sel^T @ h over 256-row
    subtile pairs, Sel exact {0,1} in fp8) accumulating into a single PSUM
    tile that lives across the whole kernel, batched every 4 macrotiles.
  - Epilogue (per core): logits[b, c] = b2[c] + sum_j sums[b, j]*w2b[b, c, j]
    via DVE multiply + ScalarE accum_out reduction.
    DMA [64, 2] f32 out; host concatenates the 8 core outputs.
  Numerics: fp8e4 inputs with f32 accumulation everywhere; measured rel err
  vs the f32 reference = 5.6e-3 in numpy simulation (absmax-relative,
  gate is 2e-2).
"""

import numpy as np
import ml_dtypes

N_BAGS = 512
N_CORES = 8
BPC = N_BAGS // N_CORES  # bags per core
D_IN = 1024
D_H = 512
KCH = D_IN // 128  # k-chunks of the contraction dim
KPAIR = KCH // 2  # DoubleRow processes two k-chunks per matmul
MACRO = 512  # rows per macrotile (one x DMA)
SUB = 128  # rows per subtile (one PSUM tile)

XSCALE = 16.0  # x quantization pre-scale (power of 2, exact)
WSCALE = 256.0  # W1 quantization pre-scale
HSCALE = 16.0  # scale carried by the fp8 h (16*relu(.))

_FP8 = ml_dtypes.float8_e4m3


def _build_nc(n_macro: int):
    import concourse.bacc as bacc
    import concourse.mybir as mybir
    from concourse.tile import TileContext

    f32 = mybir.dt.float32
    fp8 = mybir.dt.float8e4
    RELU = mybir.ActivationFunctionType.Relu
    COPY = mybir.ActivationFunctionType.Copy
    DR = mybir.MatmulPerfMode.DoubleRow

    nc = bacc.Bacc(None, target_bir_lowering=False)
    L = n_macro * MACRO
    xT = nc.dram_tensor("xT", [D_IN, L], fp8, kind="ExternalInput")
    # sel one-hot padded to 128 bag-columns per subtile (cols BPC..127 are
    # zero -> rows BPC..127 of the sums PSUM tile accumulate exact zeros)
    sel = nc.dram_tensor("sel", [n_macro, SUB, 4 * SUB], fp8, kind="ExternalInput")
    w1 = nc.dram_tensor("w1", [D_IN, D_H], fp8, kind="ExternalInput")
    b1 = nc.dram_tensor("b1", [SUB, D_H], f32, kind="ExternalInput")
    # w2b[b, c*D_H + j] = W2[j, c] / count[b] / HSCALE (mean division and the
    # fp8 h scale folded in; b is a LOCAL bag slot). Cols 2*D_H + c hold b2[c]
    # masked to the single owning core.
    w2b = nc.dram_tensor("w2b", [SUB, 2 * D_H + 2], f32, kind="ExternalInput")
    out = nc.dram_tensor("out", [SUB, 2], f32, kind="ExternalOutput")

    with TileContext(nc) as tc:
        with (
            tc.tile_pool(name="const", bufs=1) as cpool,
            tc.tile_pool(name="xp", bufs=4) as xpool,
            tc.tile_pool(name="selp", bufs=6) as selpool,
            tc.tile_pool(name="hp", bufs=6) as hpool,
            tc.tile_pool(name="pp", bufs=4, space="PSUM") as ppool,
            tc.tile_pool(name="sp", bufs=1, space="PSUM") as spool,
        ):
            # constants go on the ACT HWDGE ring (nc.scalar) so the SP ring's
            # sequencer can start issuing x-tile DMAs immediately
            w1_t = cpool.tile([128, KCH, D_H], fp8, name="w1_t")
            for k in range(KCH):
                nc.scalar.dma_start(
                    out=w1_t[:, k, :],
                    in_=w1[k * 128 : (k + 1) * 128, :],
                )
            # b1 broadcast across all 128 row-partitions (bias add on DVE,
            # keeping the PE stream free of rank-1 bias matmuls)
            b1_t = cpool.tile([SUB, D_H], f32, name="b1_t")
            nc.scalar.dma_start(out=b1_t[:], in_=b1[:])

            sums = spool.tile([SUB, D_H], f32, name="sums")

            pending = []
            first_seg = True
            for m in range(n_macro):
                x_t = xpool.tile([128, KCH, MACRO], fp8, name="x_t")
                # alternate x tiles between the two HWDGE rings so neither
                # sequencer's per-descriptor issue rate gates the early
                # macrotiles; sel rides the opposite ring
                xeng, seng = (
                    (nc.sync, nc.scalar)
                    if (m < 2 or m % 2 == 0)
                    else (nc.scalar, nc.sync)
                )
                # first macrotiles in finer pieces so the first matmuls can
                # start as soon as chunk 0 lands; halves afterwards
                if m == 0:
                    pieces = [(0, 2), (2, 2), (4, 2), (6, 2)]
                elif m <= 2:
                    pieces = [(0, 2), (2, 2), (4, 2), (6, 2)]
                else:
                    pieces = [(0, 4), (4, 4)]
                for ks, kstep in pieces:
                    xeng.dma_start(
                        out=x_t[:, ks : ks + kstep, :],
                        in_=xT[
                            ks * 128 : (ks + kstep) * 128,
                            m * MACRO : (m + 1) * MACRO,
                        ].rearrange("(k p) j -> p k j", p=128),
                    )
                sel_t = selpool.tile([SUB, 4, SUB], fp8, name="sel_t")
                seng.dma_start(out=sel_t[:], in_=sel[m].rearrange("p (s b) -> p s b", b=SUB))

                for t in range(2):  # subtile pairs within the macrotile
                    h_pair = hpool.tile(
                        [SUB, 2, D_H], fp8, name="h_pair", tag="h_pair", bufs=10
                    )
                    for u in range(2):
                        s = 2 * t + u
                        h_ps = ppool.tile([SUB, D_H], f32, name="h_ps")
                        for c in range(KPAIR):
                            nc.tensor.matmul(
                                h_ps[:],
                                lhsT=x_t[:, 2 * c : 2 * c + 2, s * SUB : (s + 1) * SUB],
                                rhs=w1_t[:, 2 * c : 2 * c + 2, :],
                                start=(c == 0),
                                stop=(c == KPAIR - 1),
                                perf_mode=DR,
                            )
                        hb = hpool.tile([SUB, D_H], f32, name="hb", tag="hb", bufs=4)
                        nc.vector.tensor_add(hb[:], h_ps[:], b1_t[:])
                        # 16*relu(x@W1+b1) = relu(hb/256) with hb = 4096*(x@W1+b1)
                        nc.scalar.activation(
                            h_pair[:, u, :], hb[:], RELU, scale=float(HSCALE / 4096.0)
                        )
                    pending.append((sel_t, t, h_pair))
                # segment matmuls batched across four macrotiles: one
                # stationary-switch region per batch instead of per pair.
                # Flush points aligned from the END so the final batch is a
                # full 8 - its first segs hide the last subtile's bias/relu
                # chain latency that a short final batch would expose.
                if (n_macro - 1 - m) % 4 == 0:
                    for i, (sel_ref, t, h_pair) in enumerate(pending):
                        nc.tensor.matmul(
                            sums[:],
                            lhsT=sel_ref[:, 2 * t : 2 * t + 2, :],
                            rhs=h_pair[:],
                            start=first_seg,
                            stop=(m == n_macro - 1 and i == len(pending) - 1),
                            perf_mode=DR,
                            skip_group_check=True,
                        )
                        first_seg = False
                    pending = []

            w2b_t = cpool.tile([SUB, 2 * D_H + 2], f32, name="w2b_t")
            nc.scalar.dma_start(out=w2b_t[:], in_=w2b[:])

            logits = cpool.tile([SUB, 2], f32, name="logits")
            for c in range(2):
                # multiply on DVE, row-reduce via ScalarE accum_out so the
                # two class reductions pipeline across engines
                scr = cpool.tile([SUB, D_H], f32, name=f"scr{c}")
                nc.vector.tensor_mul(
                    scr[:], sums[:], w2b_t[:, c * D_H : (c + 1) * D_H]
                )
                scr2 = cpool.tile([SUB, D_H], f32, name=f"scr2{c}")
                red = cpool.tile([SUB, 1], f32, name=f"red{c}")
                nc.scalar.activation(scr2[:], scr[:], COPY, accum_out=red[:])
                nc.vector.tensor_add(
                    logits[:, c : c + 1],
                    red[:],
                    w2b_t[:, 2 * D_H + c : 2 * D_H + c + 1],
                )
            nc.sync.dma_start(out=out[:], in_=logits[:])
    nc.finalize()
    return nc


def _prepare_inputs(x, ids, W1, b1, W2, b2):
    """Equal row split across cores (minimal padding); local bag slots.

    Core k gets rows [k*R, (k+1)*R). A bag straddling a core boundary gets
    partial logits on both cores; since logits are linear in the bag sums
    (the 1/count and W2 factors are folded host-side), the host overlap-adds
    the per-core outputs. b2 rides along masked to the owning core.
    Returns (in_maps, n_macro, first_bag, nloc) for the gather.
    """
    ids = np.asarray(ids).astype(np.int64)
    x = np.asarray(x, dtype=np.float32)
    n = x.shape[0]

    R = -(-n // N_CORES)  # rows per core
    n_macro = max(1, -(-R // MACRO))
    L = n_macro * MACRO

    counts = np.bincount(ids, minlength=N_BAGS).astype(np.float64)
    recip_all = np.where(counts > 0, 1.0 / counts, 0.0).astype(np.float32)
    first_occ = np.searchsorted(ids, np.arange(N_BAGS))  # first row of each bag

    x_f8 = (x * np.float32(XSCALE)).astype(_FP8)
    w1_f8 = (np.asarray(W1, dtype=np.float32) * np.float32(WSCALE)).astype(_FP8)
    # device computes hb = XSCALE*WSCALE*(x@W1) + bias ; bias must equal
    # XSCALE*WSCALE*b1 so relu(hb*HSCALE/(XSCALE*WSCALE)) = HSCALE*relu(x@W1+b1)
    b1_bc = np.ascontiguousarray(
        np.broadcast_to(
            (np.asarray(b1, dtype=np.float32) * np.float32(XSCALE * WSCALE))[None, :],
            (SUB, D_H),
        )
    )
    W2f = np.asarray(W2, dtype=np.float32)
    b2f = np.asarray(b2, dtype=np.float32).reshape(2)

    in_maps = []
    first_bag = np.zeros(N_CORES, dtype=np.int64)
    nloc = np.zeros(N_CORES, dtype=np.int64)
    for k in range(N_CORES):
        lo, hi = k * R, min((k + 1) * R, n)
        nk = hi - lo
        xT_k = np.zeros((D_IN, L), dtype=_FP8)
        if nk:
            xT_k[:, :nk] = x_f8[lo:hi].T

        g0 = int(ids[lo]) if nk else 0
        first_bag[k] = g0
        sel_k = np.zeros((n_macro, SUB, 4 * SUB), dtype=_FP8)
        if nk:
            r = np.arange(nk)
            lb = ids[lo:hi] - g0  # local bag slot
            assert lb.max() < SUB, "core spans more than 128 bags"
            nloc[k] = int(lb.max()) + 1
            mi = r // MACRO
            pi = r % SUB
            si = (r % MACRO) // SUB
            sel_k[mi, pi, si * SUB + lb] = 1.0

        # local slot b -> global bag g0+b (slots beyond nloc stay zero)
        w2b_k = np.zeros((SUB, 2 * D_H + 2), dtype=np.float32)
        nl = int(nloc[k])
        gl = np.arange(g0, min(g0 + nl, N_BAGS))
        rk = recip_all[gl] / np.float32(HSCALE)  # [nl]
        for c in range(2):
            w2b_k[: len(gl), c * D_H : (c + 1) * D_H] = (
                rk[:, None] * W2f[:, c][None, :]
            )
            # b2 applied only by the core owning the bag's first row
            own = (first_occ[gl] >= lo) & (first_occ[gl] < hi)
            w2b_k[: len(gl), 2 * D_H + c] = np.where(own, b2f[c], 0.0)

        in_maps.append(
            {
                "xT": xT_k,
                "sel": sel_k,
                "w1": w1_f8,
                "b1": b1_bc,
                "w2b": w2b_k,
            }
        )
    return in_maps, n_macro, first_bag, nloc


def _run(x, ids, W1, b1, W2, b2, trace=False, trace_kwargs=None):
    from concourse.bass_utils import run_bass_kernel_spmd

    in_maps, n_macro, first_bag, nloc = _prepare_inputs(x, ids, W1, b1, W2, b2)
    nc = _build_nc(n_macro)
    res = run_bass_kernel_spmd(
        nc,
        in_maps,
        list(range(N_CORES)),
        trace=trace,
        **(trace_kwargs or {}),
    )
    full = np.zeros((N_BAGS, 2), dtype=np.float32)
    for k in range(N_CORES):
        out_k = np.asarray(res.results[k]["out"], dtype=np.float32)
        g0, nl = int(first_bag[k]), int(nloc[k])
        nl = min(nl, N_BAGS - g0)
        full[g0 : g0 + nl] += out_k[:nl]
    return full, res


def kernel(x, ids, W1, b1, W2, b2):
    out, _ = _run(x, ids, W1, b1, W2, b2, trace=False)
    return out


# revision 9
# speedup vs baseline: 2.0018x; 1.1184x over previous
"""Trainium2 Bass kernel for nn_BagModel (segment_reduce).

Model: h = relu(x @ W1 + b1); bag_feat = segment_mean(h, ids); out = bag_feat @ W2 + b2
  x [262144, 1024] f32, ids [262144] int64 (sorted, 512 bags), W1 [1024, 512],
  b1 [512], W2 [512, 2], b2 [2]  ->  out [512, 2] f32

Strategy (8 NeuronCores, data-parallel over equal row ranges):
  - Host: split rows EQUALLY across cores (262144/8 = 64 macrotiles exactly,
    zero padding). Bags straddling a core boundary produce partial logits on
    both cores; logits are linear in the bag sums (1/count and W2 folded
    host-side into w2b, b2 masked to the owning core), so the host
    overlap-adds per-core outputs. Pre-transpose each shard to xT [1024, L]
    fp8e4 (x scaled by 16; W1 scaled by 256 - both powers of two, folded
    back out via the relu scale and w2b) so each 128-row subtile's x-chunk
    is directly the stationary (lhsT) matmul operand.
  - Device, per 128-row subtile: 4 fp8 DoubleRow accumulating matmuls
    (256-deep k-pairs; 2 fp8 weights/cell -> ~1.8x PE throughput vs bf16)
    into PSUM; DVE adds a row-broadcast 4096*b1; ScalarE relu(x/256) emits
    16*relu(.) as fp8e4 into one half of a subtile-pair tile.
    Segment sums = one-hot DoubleRow matmuls (Sel^T @ h over 256-row
    subtile pairs, Sel exact {0,1} in fp8) accumulating into a single PSUM
    tile that lives across the whole kernel, batched every 4 macrotiles.
  - Epilogue (per core): logits[b, c] = b2[c] + sum_j sums[b, j]*w2b[b, c, j]
    via DVE multiply + ScalarE accum_out reduction.
    DMA [64, 2] f32 out; host concatenates the 8 core outputs.
  Numerics: fp8e4 inputs with f32 accumulation everywhere; measured rel err
  vs the f32 reference = 5.6e-3 in numpy simulation (absmax-relative,
  gate is 2e-2).
"""

import numpy as np
import ml_dtypes

N_BAGS = 512
N_CORES = 8
BPC = N_BAGS // N_CORES  # bags per core
D_IN = 1024
D_H = 512
KCH = D_IN // 128  # k-chunks of the contraction dim
KPAIR = KCH // 2  # DoubleRow processes two k-chunks per matmul
MACRO = 512  # rows per macrotile (one x DMA)
SUB = 128  # rows per subtile (one PSUM tile)

XSCALE = 16.0  # x quantization pre-scale (power of 2, exact)
WSCALE = 256.0  # W1 quantization pre-scale
HSCALE = 16.0  # scale carried by the fp8 h (16*relu(.))

_FP8 = ml_dtypes.float8_e4m3


def _build_nc(n_macro: int):
    import concourse.bacc as bacc
    import concourse.mybir as mybir
    from concourse.tile import TileContext

    f32 = mybir.dt.float32
    fp8 = mybir.dt.float8e4
    RELU = mybir.ActivationFunctionType.Relu
    COPY = mybir.ActivationFunctionType.Copy
    DR = mybir.MatmulPerfMode.DoubleRow

    nc = bacc.Bacc(None, target_bir_lowering=False)
    L = n_macro * MACRO
    xT = nc.dram_tensor("xT", [D_IN, L], fp8, kind="ExternalInput")
    # sel one-hot padded to 128 bag-columns per subtile (cols BPC..127 are
    # zero -> rows BPC..127 of the sums PSUM tile accumulate exact zeros)
    sel = nc.dram_tensor("sel", [n_macro, SUB, 4 * SUB], fp8, kind="ExternalInput")
    w1 = nc.dram_tensor("w1", [D_IN, D_H], fp8, kind="ExternalInput")
    b1 = nc.dram_tensor("b1", [SUB, 2 * D_H], f32, kind="ExternalInput")
    # w2b[b, c*D_H + j] = W2[j, c] / count[b] / HSCALE (mean division and the
    # fp8 h scale folded in; b is a LOCAL bag slot). Cols 2*D_H + c hold b2[c]
    # masked to the single owning core.
    w2b = nc.dram_tensor("w2b", [SUB, 2 * D_H + 2], f32, kind="ExternalInput")
    out = nc.dram_tensor("out", [SUB, 2], f32, kind="ExternalOutput")

    with TileContext(nc) as tc:
        with (
            tc.tile_pool(name="const", bufs=1) as cpool,
            tc.tile_pool(name="xp", bufs=4) as xpool,
            tc.tile_pool(name="selp", bufs=8) as selpool,
            tc.tile_pool(name="hp", bufs=6) as hpool,
            tc.tile_pool(name="pp", bufs=3, space="PSUM") as ppool,
            tc.tile_pool(name="sp", bufs=1, space="PSUM") as spool,
        ):
            # constants go on the ACT HWDGE ring (nc.scalar) so the SP ring's
            # sequencer can start issuing x-tile DMAs immediately
            w1_t = cpool.tile([128, KCH, D_H], fp8, name="w1_t")
            for k in range(KCH):
                nc.scalar.dma_start(
                    out=w1_t[:, k, :],
                    in_=w1[k * 128 : (k + 1) * 128, :],
                )
            # b1 broadcast across all 128 row-partitions and duplicated twice
            # along free so one DVE add covers a whole subtile pair (bias add
            # on DVE, keeping the PE stream free of rank-1 bias matmuls)
            b1_t = cpool.tile([SUB, 2, D_H], f32, name="b1_t")
            nc.scalar.dma_start(
                out=b1_t[:], in_=b1[:].rearrange("p (i d) -> p i d", d=D_H)
            )

            sums = spool.tile([SUB, D_H], f32, name="sums")

            pending = []
            first_seg = True
            for m in range(n_macro):
                x_t = xpool.tile([128, KCH, MACRO], fp8, name="x_t")
                # alternate x tiles between the two HWDGE rings so neither
                # sequencer's per-descriptor issue rate gates the early
                # macrotiles; sel rides the opposite ring
                xeng, seng = (
                    (nc.sync, nc.scalar)
                    if (m < 2 or m % 2 == 0)
                    else (nc.scalar, nc.sync)
                )
                # first macrotiles in finer pieces so the first matmuls can
                # start as soon as chunk 0 lands; halves afterwards
                if m == 0:
                    pieces = [(0, 2), (2, 2), (4, 2), (6, 2)]
                elif m <= 2:
                    pieces = [(0, 2), (2, 2), (4, 2), (6, 2)]
                else:
                    pieces = [(0, 4), (4, 4)]
                for ks, kstep in pieces:
                    xeng.dma_start(
                        out=x_t[:, ks : ks + kstep, :],
                        in_=xT[
                            ks * 128 : (ks + kstep) * 128,
                            m * MACRO : (m + 1) * MACRO,
                        ].rearrange("(k p) j -> p k j", p=128),
                    )
                sel_t = selpool.tile([SUB, 4, SUB], fp8, name="sel_t")
                seng.dma_start(out=sel_t[:], in_=sel[m].rearrange("p (s b) -> p s b", b=SUB))

                for t in range(2):  # subtile pairs within the macrotile
                    h_pair = hpool.tile(
                        [SUB, 2, D_H], fp8, name="h_pair", tag="h_pair", bufs=18
                    )
                    # both subtiles of the pair matmul into one 2-bank PSUM
                    # tile so a single DVE add / ScalarE relu covers the pair
                    # (amortizes the trn2 per-instruction read-write bubble)
                    h_ps = ppool.tile([SUB, 2, D_H], f32, name="h_ps")
                    for u in range(2):
                        s = 2 * t + u
                        for c in range(KPAIR):
                            nc.tensor.matmul(
                                h_ps[:, u, :],
                                lhsT=x_t[:, 2 * c : 2 * c + 2, s * SUB : (s + 1) * SUB],
                                rhs=w1_t[:, 2 * c : 2 * c + 2, :],
                                start=(c == 0),
                                stop=(c == KPAIR - 1),
                                perf_mode=DR,
                            )
                    hb = hpool.tile([SUB, 2, D_H], f32, name="hb", tag="hb", bufs=3)
                    nc.vector.tensor_add(hb[:], h_ps[:], b1_t[:])
                    # 16*relu(x@W1+b1) = relu(hb/256) with hb = 4096*(x@W1+b1)
                    nc.scalar.activation(
                        h_pair[:], hb[:], RELU, scale=float(HSCALE / 4096.0)
                    )
                    pending.append((sel_t, t, h_pair))
                # segment matmuls batched (8 pairs = 4 macrotiles per batch)
                # and DELAYED ~2 macrotiles behind production so the PE never
                # waits on the bias/relu chain of a just-produced h_pair; the
                # leftover pairs flush after the last macrotile (short tail).
                if len(pending) >= 12:
                    for sel_ref, t, h_pair in pending[:8]:
                        nc.tensor.matmul(
                            sums[:],
                            lhsT=sel_ref[:, 2 * t : 2 * t + 2, :],
                            rhs=h_pair[:],
                            start=first_seg,
                            stop=False,
                            perf_mode=DR,
                            skip_group_check=True,
                        )
                        first_seg = False
                    pending = pending[8:]
            for i, (sel_ref, t, h_pair) in enumerate(pending):
                nc.tensor.matmul(
                    sums[:],
                    lhsT=sel_ref[:, 2 * t : 2 * t + 2, :],
                    rhs=h_pair[:],
                    start=first_seg,
                    stop=(i == len(pending) - 1),
                    perf_mode=DR,
                    skip_group_check=True,
                )
                first_seg = False
            pending = []

            w2b_t = cpool.tile([SUB, 2 * D_H + 2], f32, name="w2b_t")
            nc.scalar.dma_start(out=w2b_t[:], in_=w2b[:])

            logits = cpool.tile([SUB, 2], f32, name="logits")
            # one DVE multiply covers both classes; per-class row-reduce via
            # ScalarE accum_out; one DVE add applies the (masked) b2
            scr = cpool.tile([SUB, 2, D_H], f32, name="scr")
            nc.vector.tensor_mul(
                scr[:],
                sums[:].unsqueeze(1).broadcast_to([SUB, 2, D_H]),
                w2b_t[:, : 2 * D_H].rearrange("p (i d) -> p i d", d=D_H),
            )
            scr2 = cpool.tile([SUB, D_H], f32, name="scr2")
            red = cpool.tile([SUB, 2], f32, name="red")
            for c in range(2):
                nc.scalar.activation(
                    scr2[:], scr[:, c, :], COPY, accum_out=red[:, c : c + 1]
                )
            nc.vector.tensor_add(logits[:], red[:], w2b_t[:, 2 * D_H : 2 * D_H + 2])
            nc.sync.dma_start(out=out[:], in_=logits[:])
    nc.finalize()
    return nc


def _prepare_inputs(x, ids, W1, b1, W2, b2):
    """Equal row split across cores (minimal padding); local bag slots.

    Core k gets rows [k*R, (k+1)*R). A bag straddling a core boundary gets
    partial logits on both cores; since logits are linear in the bag sums
    (the 1/count and W2 factors are folded host-side), the host overlap-adds
    the per-core outputs. b2 rides along masked to the owning core.
    Returns (in_maps, n_macro, first_bag, nloc) for the gather.
    """
    ids = np.asarray(ids).astype(np.int64)
    x = np.asarray(x, dtype=np.float32)
    n = x.shape[0]

    R = -(-n // N_CORES)  # rows per core
    n_macro = max(1, -(-R // MACRO))
    L = n_macro * MACRO

    counts = np.bincount(ids, minlength=N_BAGS).astype(np.float64)
    recip_all = np.where(counts > 0, 1.0 / counts, 0.0).astype(np.float32)
    first_occ = np.searchsorted(ids, np.arange(N_BAGS))  # first row of each bag

    x_f8 = (x * np.float32(XSCALE)).astype(_FP8)
    w1_f8 = (np.asarray(W1, dtype=np.float32) * np.float32(WSCALE)).astype(_FP8)
    # device computes hb = XSCALE*WSCALE*(x@W1) + bias ; bias must equal
    # XSCALE*WSCALE*b1 so relu(hb*HSCALE/(XSCALE*WSCALE)) = HSCALE*relu(x@W1+b1)
    b1_bc = np.ascontiguousarray(
        np.broadcast_to(
            np.tile(
                np.asarray(b1, dtype=np.float32) * np.float32(XSCALE * WSCALE), 2
            )[None, :],
            (SUB, 2 * D_H),
        )
    )
    W2f = np.asarray(W2, dtype=np.float32)
    b2f = np.asarray(b2, dtype=np.float32).reshape(2)

    in_maps = []
    first_bag = np.zeros(N_CORES, dtype=np.int64)
    nloc = np.zeros(N_CORES, dtype=np.int64)
    for k in range(N_CORES):
        lo, hi = k * R, min((k + 1) * R, n)
        nk = hi - lo
        xT_k = np.zeros((D_IN, L), dtype=_FP8)
        if nk:
            xT_k[:, :nk] = x_f8[lo:hi].T

        g0 = int(ids[lo]) if nk else 0
        first_bag[k] = g0
        sel_k = np.zeros((n_macro, SUB, 4 * SUB), dtype=_FP8)
        if nk:
            r = np.arange(nk)
            lb = ids[lo:hi] - g0  # local bag slot
            assert lb.max() < SUB, "core spans more than 128 bags"
            nloc[k] = int(lb.max()) + 1
            mi = r // MACRO
            pi = r % SUB
            si = (r % MACRO) // SUB
            sel_k[mi, pi, si * SUB + lb] = 1.0

        # local slot b -> global bag g0+b (slots beyond nloc stay zero)
        w2b_k = np.zeros((SUB, 2 * D_H + 2), dtype=np.float32)
        nl = int(nloc[k])
        gl = np.arange(g0, min(g0 + nl, N_BAGS))
        rk = recip_all[gl] / np.float32(HSCALE)  # [nl]
        for c in range(2):
            w2b_k[: len(gl), c * D_H : (c + 1) * D_H] = (
                rk[:, None] * W2f[:, c][None, :]
            )
            # b2 applied only by the core owning the bag's first row
            own = (first_occ[gl] >= lo) & (first_occ[gl] < hi)
            w2b_k[: len(gl), 2 * D_H + c] = np.where(own, b2f[c], 0.0)

        in_maps.append(
            {
                "xT": xT_k,
                "sel": sel_k,
                "w1": w1_f8,
                "b1": b1_bc,
                "w2b": w2b_k,
            }
        )
    return in_maps, n_macro, first_bag, nloc


def _run(x, ids, W1, b1, W2, b2, trace=False, trace_kwargs=None):
    from concourse.bass_utils import run_bass_kernel_spmd

    in_maps, n_macro, first_bag, nloc = _prepare_inputs(x, ids, W1, b1, W2, b2)
    nc = _build_nc(n_macro)
    res = run_bass_kernel_spmd(
        nc,
        in_maps,
        list(range(N_CORES)),
        trace=trace,
        **(trace_kwargs or {}),
    )
    full = np.zeros((N_BAGS, 2), dtype=np.float32)
    for k in range(N_CORES):
        out_k = np.asarray(res.results[k]["out"], dtype=np.float32)
        g0, nl = int(first_bag[k]), int(nloc[k])
        nl = min(nl, N_BAGS - g0)
        full[g0 : g0 + nl] += out_k[:nl]
    return full, res


def kernel(x, ids, W1, b1, W2, b2):
    out, _ = _run(x, ids, W1, b1, W2, b2, trace=False)
    return out
